# revision 55
# baseline (speedup 1.0000x reference)
# Trainium2 Bass kernel for nn_ExpandFrame: gaussian-upsampling attention
#   e = cumsum(duration, -1); c = e - 0.5*round(duration)
#   logits[b,n,t] = temp * (t - c[b,n])^2 ;  temp = -1/(5*sqrt(duration[0,0]))
#   w = softmax(logits, axis=n) ;  out[b,d,t] = sum_n w[b,n,t] * hidden[b,n,d]
#
# v2 strategy (data-parallel over batch, 2 batches/core):
#  - hidden and the output travel as bf16 (host converts both ways): DMA
#    traffic drops from 42MB to 21MB per core.
#  - softmax numerator exp(-s2*(t-c_n)^2) is computed directly in the
#    matmul's [n_partition, t_free] layout: one Square+Exp ACT pass per
#    n-chunk over that chunk's full active t-range (the weight band is
#    narrow, ~680 t per 128-n chunk). No PE transposes needed.
#  - softmax denominator 1/S is computed on the host (exact, stable) and
#    applied on-device: rb[p,t]=r[t] is materialized per 512-t block by a
#    tiny PE matmul (selector x r-row), then p_norm = p * rb on DVE.
#  - banded matmuls accumulate over the 1-2 n-chunks per 256-t half into
#    512-t PSUM banks; PSUM->SBUF bf16 casts rotate over ACT/DVE/Pool.
#  - right-tail t (beyond the last center, where softmax needs max-
#    subtraction) is handled by a per-128-chunk t-layout path with the
#    host-provided per-t min bias, then PE-transposed into p.
import numpy as np

B, N, D, T = 16, 1024, 1024, 4096
NCORES = 8
BPC = B // NCORES        # batches per core
P = 128                  # partitions
KN = N // P              # 8 n-chunks
TC = 128                 # stabilized-chunk granularity
NTC = T // TC            # 32
HB = 256                 # matmul half-tile (window granularity)
NHB = T // HB            # 16
BANK = 512               # rb PSUM width
NBK = T // BANK          # 8
TG = 2048                # output-DMA group width
NTG = T // TG            # 2
ST_CUTS = [0, 640, 1664, 2688, 3712, T]   # st ramp DMA slice boundaries
STB = 1024               # st iota chunk stride
STSPAN = 2560            # st iota chunk span (covers any PR window)
MARGIN_Q = 60.0          # band cut: keep n with q - min_q <= MARGIN_Q


class Geom:
    pass


def _host_prep(duration):
    import ml_dtypes

    dur = np.asarray(duration, dtype=np.float64)
    e = np.cumsum(dur, axis=-1)
    c = e - 0.5 * np.round(dur)                     # [B, N] f64
    d00 = float(np.asarray(duration)[0, 0])
    s2 = 1.0 / (5.0 * np.sqrt(d00))
    s = float(np.sqrt(s2))
    margin = int(np.ceil(np.sqrt(MARGIN_Q / s2))) + 2
    tgrid = np.arange(T, dtype=np.float64)

    # --- per-slot program geometry (shared by all cores) ---
    geoms = []
    for slot in range(BPC):
        bs = np.arange(NCORES) * BPC + slot
        g = Geom()
        # nearest-center distance -> which 128-t chunks need stabilization
        need = np.zeros(NTC, dtype=bool)
        for b in bs:
            idx = np.searchsorted(c[b], tgrid)
            dl = np.abs(tgrid - c[b][np.clip(idx - 1, 0, N - 1)])
            dr = np.abs(c[b][np.clip(idx, 0, N - 1)] - tgrid)
            dmin = np.minimum(dl, dr)
            need |= ((s2 * dmin * dmin).reshape(NTC, TC).max(axis=1) > 25.0)
        if need.any():
            tc0 = int(np.argmax(need))
            g.tail_lo = tc0 * TC
            g.tail_chunks = list(range(tc0, NTC))
        else:
            g.tail_lo = T
            g.tail_chunks = []
        # tail window chunks (n-chunks feeding tail t)
        if g.tail_chunks:
            n_lo = min(int(np.searchsorted(c[b], g.tail_lo - margin)) for b in bs)
            n_lo = max(0, n_lo - 1)
            g.ktail = list(range(n_lo // P, KN))
        else:
            g.ktail = []
        # per-n-chunk active t ranges
        g.PR = []        # normal (ACT-written) range, clipped to tail_lo
        g.PEx = []       # p-tile extent (incl tail for ktail chunks)
        g.AL = []        # 256-aligned extents (pn tiles / matmul windows)
        for k in range(KN):
            plo = min(c[b][k * P] for b in bs) - margin
            phi = max(c[b][k * P + P - 1] for b in bs) + margin
            plo = int(np.clip(np.floor(plo), 0, g.tail_lo))
            phi = int(np.clip(np.ceil(phi) + 1, 0, g.tail_lo))
            plo = min(plo, phi)
            pe_hi = T if k in g.ktail else phi
            al_lo = (plo // HB) * HB
            al_hi = -(-pe_hi // HB) * HB
            g.PR.append((plo, phi))
            g.PEx.append((plo, pe_hi))
            g.AL.append((al_lo, al_hi))
        # matmul windows per 256-half
        g.win = []
        for h in range(NHB):
            hlo, hhi = h * HB, (h + 1) * HB
            ks = [k for k in range(KN)
                  if g.PEx[k][0] < hhi and g.PEx[k][1] > hlo]
            assert ks, f"empty window at half {h} slot {slot}"
            g.win.append(ks)
        geoms.append(g)

    KT = max([len(g.ktail) for g in geoms] + [1])
    NTLC = max([len(g.tail_chunks) for g in geoms] + [1])
    POSW = 128
    PW = 128
    for g in geoms:
        for k in range(KN):
            POSW = max(POSW, g.PR[k][1] - g.PR[k][0])
            PW = max(PW, g.AL[k][1] - g.AL[k][0])
        POSW = max(POSW, len(g.ktail) * P)
    POSW = -(-POSW // 64) * 64
    PW = -(-PW // 64) * 64

    # --- per-batch input arrays ---
    W = margin + 8
    offs = np.arange(-W, W + 1)
    r_all = np.empty((B, T), dtype=np.float32)
    mtail = np.zeros((B, NTLC, P), dtype=np.float32)
    cbw = np.zeros((B, 1, KT * P), dtype=np.float32)
    cTn = np.empty((B, P, KN), dtype=np.float32)
    for b in range(B):
        g = geoms[b % BPC]
        idx = np.searchsorted(c[b], tgrid)
        ni = idx[:, None] + offs[None, :]
        valid = (ni >= 0) & (ni <= N - 1)
        cg = c[b][np.clip(ni, 0, N - 1)]
        q = s2 * (tgrid[:, None] - cg) ** 2
        qm = np.where(valid, q, np.inf)
        m = qm.min(axis=1)
        K = np.where(tgrid >= g.tail_lo, m, 0.0)
        S = np.where(valid, np.exp(K[:, None] - q), 0.0).sum(axis=1)
        r_all[b] = (1.0 / S).astype(np.float32)
        for j, tc in enumerate(g.tail_chunks):
            mtail[b, j] = m[tc * TC:(tc + 1) * TC].astype(np.float32)
        if g.ktail:
            nk = len(g.ktail)
            cbw[b, 0, :nk * P] = c[b][g.ktail[0] * P:].astype(np.float32)
        cTn[b] = (-s * c[b].reshape(KN, P).T).astype(np.float32)
    rr = r_all.reshape(B, NBK, BANK)
    e8 = np.zeros((NBK, NBK * P), dtype=np.float32)
    for j in range(NBK):
        e8[j, j * P:(j + 1) * P] = 1.0

    consts = dict(s=s, s2=s2, margin=margin, KT=KT, NTLC=NTLC,
                  POSW=POSW, PW=PW)
    st16 = np.tile(np.arange(T, dtype=np.int16), (P, 1))
    arrays = dict(cTn=cTn, rr=rr, cbw=cbw, mtail=mtail, e8=e8, st16=st16)
    return geoms, consts, arrays


def _build(nc, geoms, consts):
    import contextlib

    import concourse.mybir as mybir
    import concourse.tile as tile
    from concourse import masks

    f32 = mybir.dt.float32
    i32 = mybir.dt.int32
    bf16 = mybir.dt.bfloat16
    AF = mybir.ActivationFunctionType
    ALU = mybir.AluOpType
    s = consts["s"]
    KT, NTLC = consts["KT"], consts["NTLC"]
    POSW, PW = consts["POSW"], consts["PW"]

    hid_d = nc.dram_tensor("hidden", [BPC, N, D], bf16, kind="ExternalInput").ap()
    cTn_d = nc.dram_tensor("cTn", [BPC, P, KN], f32, kind="ExternalInput").ap()
    rr_d = nc.dram_tensor("rr", [BPC, NBK, BANK], f32, kind="ExternalInput").ap()
    e8_d = nc.dram_tensor("e8", [NBK, NBK * P], f32, kind="ExternalInput").ap()
    cbw_d = nc.dram_tensor("cbw", [BPC, 1, KT * P], f32, kind="ExternalInput").ap()
    mt_d = nc.dram_tensor("mtail", [BPC, NTLC, P], f32, kind="ExternalInput").ap()
    i16 = mybir.dt.int16
    st_d = nc.dram_tensor("st16", [P, T], i16, kind="ExternalInput").ap()
    out_d = nc.dram_tensor("out", [BPC, D, T], bf16, kind="ExternalOutput").ap()

    with tile.TileContext(nc) as tc:
        with contextlib.ExitStack() as ctx:
            constp = ctx.enter_context(tc.tile_pool(name="const", bufs=1))
            hidp = ctx.enter_context(tc.tile_pool(name="hid", bufs=2))
            auxp = ctx.enter_context(tc.tile_pool(name="aux", bufs=2))
            posp = ctx.enter_context(tc.tile_pool(name="pos", bufs=3))
            pp = ctx.enter_context(tc.tile_pool(name="p", bufs=2 * KN))
            pnp = ctx.enter_context(tc.tile_pool(name="pn", bufs=2 * KN))
            cbp = ctx.enter_context(tc.tile_pool(name="cb", bufs=2))
            tlp = ctx.enter_context(tc.tile_pool(name="tl", bufs=2))
            osbp = ctx.enter_context(tc.tile_pool(name="osb", bufs=5))
            pop = ctx.enter_context(tc.tile_pool(name="po", bufs=4, space="PSUM"))
            rbp = ctx.enter_context(tc.tile_pool(name="rb", bufs=3, space="PSUM"))
            ptp = ctx.enter_context(tc.tile_pool(name="pt", bufs=1, space="PSUM"))

            # ---- constants ----
            ident = constp.tile([P, P], bf16)
            masks.make_identity(nc, ident[:])
            # e8[c, j*128+p] = (c == j): selector rows for rb broadcast
            f32r = mybir.dt.float32r
            e8 = constp.tile([NBK, NBK * P], f32r)
            nc.sync.dma_start(e8[:], e8_d.bitcast(f32r))

            # st[p, t] = t (ACT input ramp; int16 exact for t < 32768).
            # DMA'd from host in slices interleaved with batch-0's input
            # DMAs (see emit_input_dmas) so the first exp waits only ~1.
            st_t = constp.tile([P, T], i16)

            def emit_st_dma(j):
                lo, hi = ST_CUTS[j], ST_CUTS[j + 1]
                nc.sync.dma_start(st_t[:, lo:hi], st_d[:, lo:hi])

            def st_slice(lo, hi):
                return st_t[:, lo:hi]
            # tneg[p, tc] = -s * (tc*128 + p)  (tail-path per-t bias)
            tneg_i = constp.tile([P, NTC], i32)
            nc.gpsimd.iota(tneg_i[:], pattern=[[P, NTC]], base=0,
                           channel_multiplier=1)
            tneg_f = constp.tile([P, NTC], f32)
            nc.scalar.mul(tneg_f[:], tneg_i[:], -s)
            # warm ACT spline tables before the DMA flood
            warm = constp.tile([P, 1], f32)
            nc.scalar.activation(warm[:], tneg_f[:, 0:1], AF.Square,
                                 bias=0.0, scale=1.0)
            nc.scalar.activation(warm[:], warm[:], AF.Exp, bias=0.0, scale=-1.0)

            def emit_input_dmas(b):
                # aux first: the first exp only needs cTn, not the hid flood.
                # For b=0 the st ramp slices interleave with the hid chunks.
                tiles = {}
                cTn_t = auxp.tile([P, KN], f32, tag="cTn")
                nc.sync.dma_start(cTn_t[:], cTn_d[b])
                tiles["cTn"] = cTn_t
                rr_t = auxp.tile([NBK, BANK], f32r, tag="rr")
                nc.sync.dma_start(rr_t[:], rr_d[b].bitcast(f32r))
                tiles["rr"] = rr_t
                if b == 0:
                    emit_st_dma(0)
                g = geoms[b]
                hid_t = hidp.tile([P, KN, D], bf16, tag="hid")
                for k in range(KN):
                    nc.sync.dma_start(hid_t[:, k, :], hid_d[b, k * P:(k + 1) * P, :])
                    if b == 0 and k in (0, 1, 3, 5):
                        emit_st_dma({0: 1, 1: 2, 3: 3, 5: 4}[k])
                tiles["hid"] = hid_t
                if g.tail_chunks:
                    cbw_t = auxp.tile([1, KT * P], f32, tag="cbw")
                    nc.sync.dma_start(cbw_t[:, 0:len(g.ktail) * P],
                                      cbw_d[b][:, 0:len(g.ktail) * P])
                    tiles["cbw"] = cbw_t
                    mt_t = auxp.tile([P, NTLC], f32, tag="mt")
                    for j in range(len(g.tail_chunks)):
                        nc.sync.dma_start(mt_t[:, j:j + 1], mt_d[b, j][:, None])
                    tiles["mt"] = mt_t
                return tiles

            # ---------- per-batch state ----------
            cur = [None] * BPC         # input tiles
            p_t = [None] * BPC         # p tiles per k
            pn_t = [None] * BPC        # p_norm tiles per k
            rb_t = [dict() for _ in range(BPC)]
            steps = [None] * BPC       # softmax step closures
            nstep = [0] * BPC          # steps emitted so far

            def mk_steps(b):
                """Ordered softmax-phase steps for batch b. Order: exp_k
                (ACT) with pn_k trailing one behind (DVE), tail path before
                pn of the tail window chunks."""
                g = geoms[b]

                def e_step(k):
                    def run():
                        plo, phi = g.PR[k]
                        al_lo, al_hi = g.AL[k]
                        pk = pp.tile([P, PW], bf16, tag="p")
                        if phi > plo:
                            pos = posp.tile([P, POSW], f32, tag="pos")
                            if b == 0 and k <= 3:
                                # startup ladder: square on DVE so the ACT
                                # exp chain parallelizes with it
                                nc.vector.tensor_scalar(
                                    pos[:, :phi - plo], st_slice(plo, phi),
                                    s, cur[b]["cTn"][:, k:k + 1],
                                    op0=ALU.mult, op1=ALU.add)
                                nc.vector.tensor_tensor(
                                    pos[:, :phi - plo], pos[:, :phi - plo],
                                    pos[:, :phi - plo], op=ALU.mult)
                            else:
                                nc.scalar.activation(
                                    pos[:, :phi - plo], st_slice(plo, phi),
                                    AF.Square, bias=cur[b]["cTn"][:, k:k + 1],
                                    scale=s)
                            nc.scalar.activation(
                                pk[:, plo - al_lo:phi - al_lo],
                                pos[:, :phi - plo], AF.Exp, bias=0.0,
                                scale=-1.0)
                        if plo > al_lo:
                            nc.gpsimd.memset(pk[:, 0:plo - al_lo], 0.0)
                        pe_hi = g.PEx[k][1]
                        if al_hi > pe_hi:
                            nc.gpsimd.memset(
                                pk[:, pe_hi - al_lo:al_hi - al_lo], 0.0)
                        p_t[b][k] = pk
                    return run

                def tail_step():
                    def run():
                        nk = len(g.ktail)
                        cbs = cbp.tile([P, KT * P], f32, tag="cbs")
                        nc.gpsimd.partition_broadcast(
                            cbs[:, 0:nk * P], cur[b]["cbw"][:, 0:nk * P],
                            channels=P)
                        for j, tcid in enumerate(g.tail_chunks):
                            post = posp.tile([P, POSW], f32, tag="pos")
                            nc.scalar.activation(
                                post[:, :nk * P], cbs[:, 0:nk * P], AF.Square,
                                bias=tneg_f[:, tcid:tcid + 1], scale=s)
                            ptt = tlp.tile([P, KT * P], bf16, tag="ptt")
                            nc.scalar.activation(
                                ptt[:, :nk * P], post[:, :nk * P], AF.Exp,
                                bias=cur[b]["mt"][:, j:j + 1], scale=-1.0)
                            for ki, k in enumerate(g.ktail):
                                pt_ps = ptp.tile([P, P], f32, tag="pt")
                                nc.tensor.matmul(pt_ps[:],
                                                 ptt[:, ki * P:(ki + 1) * P],
                                                 ident[:], start=True, stop=True)
                                al_lo = g.AL[k][0]
                                nc.vector.tensor_copy(
                                    p_t[b][k][:, tcid * TC - al_lo:
                                              tcid * TC + TC - al_lo], pt_ps[:])
                    return run

                def pn_step(k):
                    def run():
                        al_lo, al_hi = g.AL[k]
                        pnk = pnp.tile([P, PW], bf16, tag="pn")
                        j0, j1 = al_lo // BANK, -(-al_hi // BANK)
                        for j in range(j0, j1):
                            lo = max(al_lo, j * BANK)
                            hi = min(al_hi, (j + 1) * BANK)
                            if j not in rb_t[b]:
                                rb = rbp.tile([P, BANK], f32, tag="rb")
                                nc.tensor.matmul(rb[:], e8[:, j * P:(j + 1) * P],
                                                 cur[b]["rr"][:],
                                                 start=True, stop=True)
                                rb_t[b][j] = rb
                            rb = rb_t[b][j]
                            nc.vector.tensor_tensor(
                                pnk[:, lo - al_lo:hi - al_lo],
                                p_t[b][k][:, lo - al_lo:hi - al_lo],
                                rb[:, lo - j * BANK:hi - j * BANK], op=ALU.mult)
                        pn_t[b][k] = pnk
                    return run

                lst = []
                pn_idx = {}
                ktail = set(g.ktail)
                # exp steps with pn trailing by one; tail before pn of ktail
                pend_pn = []
                for k in range(KN):
                    lst.append(e_step(k))
                    pend_pn.append(k)
                    while len(pend_pn) > 1:
                        kk = pend_pn[0]
                        if kk in ktail:
                            break
                        pn_idx[kk] = len(lst)
                        lst.append(pn_step(kk))
                        pend_pn.pop(0)
                if g.tail_chunks:
                    lst.append(tail_step())
                for kk in pend_pn:
                    pn_idx[kk] = len(lst)
                    lst.append(pn_step(kk))
                # step index needed before each output group tg
                req = []
                for tg in range(NTG):
                    ks = set()
                    for h in range(tg * (TG // HB), (tg + 1) * (TG // HB)):
                        ks |= set(g.win[h])
                    req.append(max(pn_idx[k] for k in ks) + 1)
                return lst, req

            def ensure_steps(b, upto):
                while nstep[b] < min(upto, len(steps[b])):
                    steps[b][nstep[b]]()
                    nstep[b] += 1

            reqs = [None] * BPC
            cur[0] = emit_input_dmas(0)
            for b in range(BPC):
                g = geoms[b]
                p_t[b] = [None] * KN
                pn_t[b] = [None] * KN
                steps[b], reqs[b] = mk_steps(b)
            # ---------- pipelined schedule ----------
            cast_seq = ["v", "v", "v", "s"]
            while len(cast_seq) < NTG * (D // P) * (TG // BANK):
                cast_seq += ["s", "v"]
            for b in range(BPC):
                g = geoms[b]
                if b + 1 < BPC:
                    cur[b + 1] = emit_input_dmas(b + 1)
                ci = 0
                gi = 0
                for tg in range(NTG):
                    ensure_steps(b, reqs[b][tg])
                    for dci in range(D // P):
                        # drip remaining softmax steps between groups:
                        # this batch's first, then the next batch's
                        if nstep[b] < len(steps[b]):
                            ensure_steps(b, nstep[b] + 1)
                        elif b + 1 < BPC and gi % 2 == 1:
                            ensure_steps(b + 1, nstep[b + 1] + 1)
                        gi += 1
                        hid_t = cur[b]["hid"]
                        osb = osbp.tile([P, TG], bf16, tag="osb")
                        for bq in range(TG // BANK):
                            po = pop.tile([P, BANK], f32, tag="po")
                            h0 = (tg * (TG // BANK) + bq) * (BANK // HB)
                            if g.win[h0] == g.win[h0 + 1]:
                                # both halves share the window: 512-wide MMs
                                ks = g.win[h0]
                                for ki, k in enumerate(ks):
                                    al_lo = g.AL[k][0]
                                    nc.tensor.matmul(
                                        po[:],
                                        hid_t[:, k, dci * P:(dci + 1) * P],
                                        pn_t[b][k][:, h0 * HB - al_lo:
                                                   (h0 + 2) * HB - al_lo],
                                        start=(ki == 0),
                                        stop=(ki == len(ks) - 1))
                            else:
                                for hl in range(BANK // HB):
                                    h = h0 + hl
                                    ks = g.win[h]
                                    for ki, k in enumerate(ks):
                                        al_lo = g.AL[k][0]
                                        nc.tensor.matmul(
                                            po[:, hl * HB:(hl + 1) * HB],
                                            hid_t[:, k, dci * P:(dci + 1) * P],
                                            pn_t[b][k][:, h * HB - al_lo:
                                                       (h + 1) * HB - al_lo],
                                            start=(ki == 0),
                                            stop=(ki == len(ks) - 1))
                            dst = osb[:, bq * BANK:(bq + 1) * BANK]
                            eng = cast_seq[ci % len(cast_seq)]
                            ci += 1
                            if eng == "s":
                                nc.scalar.copy(dst, po[:])
                            else:
                                nc.vector.tensor_copy(dst, po[:])
                        nc.sync.dma_start(
                            out_d[b, dci * P:(dci + 1) * P,
                                  tg * TG:(tg + 1) * TG], osb[:])
    return nc


def _run(inputs, trace=False):
    import ml_dtypes

    import concourse.bacc as bacc
    from concourse.bass_utils import run_bass_kernel_spmd

    hidden = np.asarray(inputs["hidden"], dtype=np.float32)
    duration = np.asarray(inputs["duration"], dtype=np.float32)

    geoms, consts, arrays = _host_prep(duration)
    hid_bf = hidden.astype(ml_dtypes.bfloat16)

    nc = bacc.Bacc("TRN2", target_bir_lowering=False, debug=False,
                   enable_asserts=False, num_devices=NCORES)
    _build(nc, geoms, consts)
    nc.compile()

    in_maps = []
    for i in range(NCORES):
        sl = slice(i * BPC, (i + 1) * BPC)
        in_maps.append({
            "hidden": np.ascontiguousarray(hid_bf[sl]),
            "cTn": np.ascontiguousarray(arrays["cTn"][sl]),
            "rr": np.ascontiguousarray(arrays["rr"][sl]),
            "cbw": np.ascontiguousarray(arrays["cbw"][sl]),
            "mtail": np.ascontiguousarray(arrays["mtail"][sl]),
            "e8": arrays["e8"],
            "st16": arrays["st16"],
        })
    res = run_bass_kernel_spmd(nc, in_maps, core_ids=list(range(NCORES)),
                               trace=trace)
    out = np.concatenate([res.results[i]["out"] for i in range(NCORES)], axis=0)
    out = out.astype(np.float32)
    return out, res


def kernel(**inputs) -> np.ndarray:
    out, _ = _run(inputs, trace=False)
    return out


# revision 60
# speedup vs baseline: 1.0157x; 1.0157x over previous
# Trainium2 Bass kernel for nn_ExpandFrame: gaussian-upsampling attention
#   e = cumsum(duration, -1); c = e - 0.5*round(duration)
#   logits[b,n,t] = temp * (t - c[b,n])^2 ;  temp = -1/(5*sqrt(duration[0,0]))
#   w = softmax(logits, axis=n) ;  out[b,d,t] = sum_n w[b,n,t] * hidden[b,n,d]
#
# v2 strategy (data-parallel over batch, 2 batches/core):
#  - hidden and the output travel as bf16 (host converts both ways): DMA
#    traffic drops from 42MB to 21MB per core.
#  - softmax numerator exp(-s2*(t-c_n)^2) is computed directly in the
#    matmul's [n_partition, t_free] layout: one Square+Exp ACT pass per
#    n-chunk over that chunk's full active t-range (the weight band is
#    narrow, ~680 t per 128-n chunk). No PE transposes needed.
#  - softmax denominator 1/S is computed on the host (exact, stable) and
#    applied on-device: rb[p,t]=r[t] is materialized per 512-t block by a
#    tiny PE matmul (selector x r-row), then p_norm = p * rb on DVE.
#  - banded matmuls accumulate over the 1-2 n-chunks per 256-t half into
#    512-t PSUM banks; PSUM->SBUF bf16 casts rotate over ACT/DVE/Pool.
#  - right-tail t (beyond the last center, where softmax needs max-
#    subtraction) is handled by a per-128-chunk t-layout path with the
#    host-provided per-t min bias, then PE-transposed into p.
import numpy as np

B, N, D, T = 16, 1024, 1024, 4096
NCORES = 8
BPC = B // NCORES        # batches per core
P = 128                  # partitions
KN = N // P              # 8 n-chunks
TC = 128                 # stabilized-chunk granularity
NTC = T // TC            # 32
HB = 256                 # matmul half-tile (window granularity)
NHB = T // HB            # 16
BANK = 512               # rb PSUM width
NBK = T // BANK          # 8
TG = 2048                # output-DMA group width
NTG = T // TG            # 2
ST_CUTS = [0, 320, 640, 1664, 2688, 3712, T]   # st ramp DMA slice boundaries
STB = 1024               # st iota chunk stride
STSPAN = 2560            # st iota chunk span (covers any PR window)
MARGIN_Q = 60.0          # band cut: keep n with q - min_q <= MARGIN_Q


class Geom:
    pass


def _host_prep(duration):
    import ml_dtypes

    dur = np.asarray(duration, dtype=np.float64)
    e = np.cumsum(dur, axis=-1)
    c = e - 0.5 * np.round(dur)                     # [B, N] f64
    d00 = float(np.asarray(duration)[0, 0])
    s2 = 1.0 / (5.0 * np.sqrt(d00))
    s = float(np.sqrt(s2))
    margin = int(np.ceil(np.sqrt(MARGIN_Q / s2))) + 2
    tgrid = np.arange(T, dtype=np.float64)

    # --- per-slot program geometry (shared by all cores) ---
    geoms = []
    for slot in range(BPC):
        bs = np.arange(NCORES) * BPC + slot
        g = Geom()
        # nearest-center distance -> which 128-t chunks need stabilization
        need = np.zeros(NTC, dtype=bool)
        for b in bs:
            idx = np.searchsorted(c[b], tgrid)
            dl = np.abs(tgrid - c[b][np.clip(idx - 1, 0, N - 1)])
            dr = np.abs(c[b][np.clip(idx, 0, N - 1)] - tgrid)
            dmin = np.minimum(dl, dr)
            need |= ((s2 * dmin * dmin).reshape(NTC, TC).max(axis=1) > 25.0)
        if need.any():
            tc0 = int(np.argmax(need))
            g.tail_lo = tc0 * TC
            g.tail_chunks = list(range(tc0, NTC))
        else:
            g.tail_lo = T
            g.tail_chunks = []
        # tail window chunks (n-chunks feeding tail t)
        if g.tail_chunks:
            n_lo = min(int(np.searchsorted(c[b], g.tail_lo - margin)) for b in bs)
            n_lo = max(0, n_lo - 1)
            g.ktail = list(range(n_lo // P, KN))
        else:
            g.ktail = []
        # per-n-chunk active t ranges
        g.PR = []        # normal (ACT-written) range, clipped to tail_lo
        g.PEx = []       # p-tile extent (incl tail for ktail chunks)
        g.AL = []        # 256-aligned extents (pn tiles / matmul windows)
        for k in range(KN):
            plo = min(c[b][k * P] for b in bs) - margin
            phi = max(c[b][k * P + P - 1] for b in bs) + margin
            plo = int(np.clip(np.floor(plo), 0, g.tail_lo))
            phi = int(np.clip(np.ceil(phi) + 1, 0, g.tail_lo))
            plo = min(plo, phi)
            pe_hi = T if k in g.ktail else phi
            al_lo = (plo // HB) * HB
            al_hi = -(-pe_hi // HB) * HB
            g.PR.append((plo, phi))
            g.PEx.append((plo, pe_hi))
            g.AL.append((al_lo, al_hi))
        # matmul windows per 256-half
        g.win = []
        for h in range(NHB):
            hlo, hhi = h * HB, (h + 1) * HB
            ks = [k for k in range(KN)
                  if g.PEx[k][0] < hhi and g.PEx[k][1] > hlo]
            assert ks, f"empty window at half {h} slot {slot}"
            g.win.append(ks)
        geoms.append(g)

    KT = max([len(g.ktail) for g in geoms] + [1])
    NTLC = max([len(g.tail_chunks) for g in geoms] + [1])
    POSW = 128
    PW = 128
    for g in geoms:
        for k in range(KN):
            POSW = max(POSW, g.PR[k][1] - g.PR[k][0])
            PW = max(PW, g.AL[k][1] - g.AL[k][0])
        POSW = max(POSW, len(g.ktail) * P)
    POSW = -(-POSW // 64) * 64
    PW = -(-PW // 64) * 64

    # --- per-batch input arrays ---
    W = margin + 8
    offs = np.arange(-W, W + 1)
    r_all = np.empty((B, T), dtype=np.float32)
    mtail = np.zeros((B, NTLC, P), dtype=np.float32)
    cbw = np.zeros((B, 1, KT * P), dtype=np.float32)
    cTn = np.empty((B, P, KN), dtype=np.float32)
    for b in range(B):
        g = geoms[b % BPC]
        idx = np.searchsorted(c[b], tgrid)
        ni = idx[:, None] + offs[None, :]
        valid = (ni >= 0) & (ni <= N - 1)
        cg = c[b][np.clip(ni, 0, N - 1)]
        q = s2 * (tgrid[:, None] - cg) ** 2
        qm = np.where(valid, q, np.inf)
        m = qm.min(axis=1)
        K = np.where(tgrid >= g.tail_lo, m, 0.0)
        S = np.where(valid, np.exp(K[:, None] - q), 0.0).sum(axis=1)
        r_all[b] = (1.0 / S).astype(np.float32)
        for j, tc in enumerate(g.tail_chunks):
            mtail[b, j] = m[tc * TC:(tc + 1) * TC].astype(np.float32)
        if g.ktail:
            nk = len(g.ktail)
            cbw[b, 0, :nk * P] = c[b][g.ktail[0] * P:].astype(np.float32)
        cTn[b] = (-s * c[b].reshape(KN, P).T).astype(np.float32)
    rr = r_all.reshape(B, NBK, BANK)
    e8 = np.zeros((NBK, NBK * P), dtype=np.float32)
    for j in range(NBK):
        e8[j, j * P:(j + 1) * P] = 1.0

    consts = dict(s=s, s2=s2, margin=margin, KT=KT, NTLC=NTLC,
                  POSW=POSW, PW=PW)
    st16 = np.tile(np.arange(T, dtype=np.int16), (P, 1))
    arrays = dict(cTn=cTn, rr=rr, cbw=cbw, mtail=mtail, e8=e8, st16=st16)
    return geoms, consts, arrays


def _build(nc, geoms, consts):
    import contextlib

    import concourse.mybir as mybir
    import concourse.tile as tile
    from concourse import masks

    f32 = mybir.dt.float32
    i32 = mybir.dt.int32
    bf16 = mybir.dt.bfloat16
    AF = mybir.ActivationFunctionType
    ALU = mybir.AluOpType
    s = consts["s"]
    KT, NTLC = consts["KT"], consts["NTLC"]
    POSW, PW = consts["POSW"], consts["PW"]

    hid_d = nc.dram_tensor("hidden", [BPC, N, D], bf16, kind="ExternalInput").ap()
    cTn_d = nc.dram_tensor("cTn", [BPC, P, KN], f32, kind="ExternalInput").ap()
    rr_d = nc.dram_tensor("rr", [BPC, NBK, BANK], f32, kind="ExternalInput").ap()
    e8_d = nc.dram_tensor("e8", [NBK, NBK * P], f32, kind="ExternalInput").ap()
    cbw_d = nc.dram_tensor("cbw", [BPC, 1, KT * P], f32, kind="ExternalInput").ap()
    mt_d = nc.dram_tensor("mtail", [BPC, NTLC, P], f32, kind="ExternalInput").ap()
    i16 = mybir.dt.int16
    st_d = nc.dram_tensor("st16", [P, T], i16, kind="ExternalInput").ap()
    out_d = nc.dram_tensor("out", [BPC, D, T], bf16, kind="ExternalOutput").ap()

    with tile.TileContext(nc) as tc:
        with contextlib.ExitStack() as ctx:
            constp = ctx.enter_context(tc.tile_pool(name="const", bufs=1))
            hidp = ctx.enter_context(tc.tile_pool(name="hid", bufs=2))
            auxp = ctx.enter_context(tc.tile_pool(name="aux", bufs=2))
            posp = ctx.enter_context(tc.tile_pool(name="pos", bufs=3))
            pp = ctx.enter_context(tc.tile_pool(name="p", bufs=2 * KN))
            pnp = ctx.enter_context(tc.tile_pool(name="pn", bufs=2 * KN))
            cbp = ctx.enter_context(tc.tile_pool(name="cb", bufs=2))
            tlp = ctx.enter_context(tc.tile_pool(name="tl", bufs=2))
            osbp = ctx.enter_context(tc.tile_pool(name="osb", bufs=5))
            pop = ctx.enter_context(tc.tile_pool(name="po", bufs=4, space="PSUM"))
            rbp = ctx.enter_context(tc.tile_pool(name="rb", bufs=3, space="PSUM"))
            ptp = ctx.enter_context(tc.tile_pool(name="pt", bufs=1, space="PSUM"))

            # ---- constants ----
            ident = constp.tile([P, P], bf16)
            masks.make_identity(nc, ident[:])
            # e8[c, j*128+p] = (c == j): selector rows for rb broadcast
            # (DMA'd inside emit_input_dmas(0), after the startup-critical
            # cTn/st slices)
            f32r = mybir.dt.float32r
            e8 = constp.tile([NBK, NBK * P], f32r)

            # st[p, t] = t (ACT input ramp; int16 exact for t < 32768).
            # DMA'd from host in slices interleaved with batch-0's input
            # DMAs (see emit_input_dmas) so the first exp waits only ~1.
            st_t = constp.tile([P, T], i16)

            def emit_st_dma(j):
                lo, hi = ST_CUTS[j], ST_CUTS[j + 1]
                nc.sync.dma_start(st_t[:, lo:hi], st_d[:, lo:hi])

            def st_slice(lo, hi):
                return st_t[:, lo:hi]
            # tneg[p, tc] = -s * (tc*128 + p)  (tail-path per-t bias)
            tneg_i = constp.tile([P, NTC], i32)
            nc.gpsimd.iota(tneg_i[:], pattern=[[P, NTC]], base=0,
                           channel_multiplier=1)
            tneg_f = constp.tile([P, NTC], f32)
            nc.scalar.mul(tneg_f[:], tneg_i[:], -s)
            # warm ACT spline tables before the DMA flood
            warm = constp.tile([P, 1], f32)
            nc.scalar.activation(warm[:], tneg_f[:, 0:1], AF.Square,
                                 bias=0.0, scale=1.0)
            nc.scalar.activation(warm[:], warm[:], AF.Exp, bias=0.0, scale=-1.0)

            def emit_input_dmas(b):
                # aux first: the first exp only needs cTn, not the hid flood.
                # For b=0 the st ramp slices interleave with the hid chunks.
                tiles = {}
                cTn_t = auxp.tile([P, KN], f32, tag="cTn")
                nc.sync.dma_start(cTn_t[:], cTn_d[b])
                tiles["cTn"] = cTn_t
                if b == 0:
                    emit_st_dma(0)
                    emit_st_dma(1)
                rr_t = auxp.tile([NBK, BANK], f32r, tag="rr")
                nc.sync.dma_start(rr_t[:], rr_d[b].bitcast(f32r))
                tiles["rr"] = rr_t
                if b == 0:
                    nc.sync.dma_start(e8[:], e8_d.bitcast(f32r))
                g = geoms[b]
                hid_t = hidp.tile([P, KN, D], bf16, tag="hid")
                for k in range(KN):
                    nc.sync.dma_start(hid_t[:, k, :], hid_d[b, k * P:(k + 1) * P, :])
                    if b == 0 and k in (0, 1, 3, 5):
                        emit_st_dma({0: 2, 1: 3, 3: 4, 5: 5}[k])
                tiles["hid"] = hid_t
                if g.tail_chunks:
                    cbw_t = auxp.tile([1, KT * P], f32, tag="cbw")
                    nc.sync.dma_start(cbw_t[:, 0:len(g.ktail) * P],
                                      cbw_d[b][:, 0:len(g.ktail) * P])
                    tiles["cbw"] = cbw_t
                    mt_t = auxp.tile([P, NTLC], f32, tag="mt")
                    for j in range(len(g.tail_chunks)):
                        nc.sync.dma_start(mt_t[:, j:j + 1], mt_d[b, j][:, None])
                    tiles["mt"] = mt_t
                return tiles

            # ---------- per-batch state ----------
            cur = [None] * BPC         # input tiles
            p_t = [None] * BPC         # p tiles per k
            pn_t = [None] * BPC        # p_norm tiles per k
            rb_t = [dict() for _ in range(BPC)]
            steps = [None] * BPC       # softmax step closures
            nstep = [0] * BPC          # steps emitted so far

            def mk_steps(b):
                """Ordered softmax-phase steps for batch b. Order: exp_k
                (ACT) with pn_k trailing one behind (DVE), tail path before
                pn of the tail window chunks."""
                g = geoms[b]

                def e_step(k):
                    def run():
                        plo, phi = g.PR[k]
                        al_lo, al_hi = g.AL[k]
                        pk = pp.tile([P, PW], bf16, tag="p")
                        if phi > plo:
                            pos = posp.tile([P, POSW], f32, tag="pos")
                            if b == 0 and 1 <= k <= 3:
                                # startup ladder: square on DVE so the ACT
                                # exp chain parallelizes with it
                                nc.vector.tensor_scalar(
                                    pos[:, :phi - plo], st_slice(plo, phi),
                                    s, cur[b]["cTn"][:, k:k + 1],
                                    op0=ALU.mult, op1=ALU.add)
                                nc.vector.tensor_tensor(
                                    pos[:, :phi - plo], pos[:, :phi - plo],
                                    pos[:, :phi - plo], op=ALU.mult)
                            else:
                                nc.scalar.activation(
                                    pos[:, :phi - plo], st_slice(plo, phi),
                                    AF.Square, bias=cur[b]["cTn"][:, k:k + 1],
                                    scale=s)
                            nc.scalar.activation(
                                pk[:, plo - al_lo:phi - al_lo],
                                pos[:, :phi - plo], AF.Exp, bias=0.0,
                                scale=-1.0)
                        if plo > al_lo:
                            nc.gpsimd.memset(pk[:, 0:plo - al_lo], 0.0)
                        pe_hi = g.PEx[k][1]
                        if al_hi > pe_hi:
                            nc.gpsimd.memset(
                                pk[:, pe_hi - al_lo:al_hi - al_lo], 0.0)
                        p_t[b][k] = pk
                    return run

                def tail_step():
                    def run():
                        nk = len(g.ktail)
                        cbs = cbp.tile([P, KT * P], f32, tag="cbs")
                        nc.gpsimd.partition_broadcast(
                            cbs[:, 0:nk * P], cur[b]["cbw"][:, 0:nk * P],
                            channels=P)
                        for j, tcid in enumerate(g.tail_chunks):
                            post = posp.tile([P, POSW], f32, tag="pos")
                            nc.scalar.activation(
                                post[:, :nk * P], cbs[:, 0:nk * P], AF.Square,
                                bias=tneg_f[:, tcid:tcid + 1], scale=s)
                            ptt = tlp.tile([P, KT * P], bf16, tag="ptt")
                            nc.scalar.activation(
                                ptt[:, :nk * P], post[:, :nk * P], AF.Exp,
                                bias=cur[b]["mt"][:, j:j + 1], scale=-1.0)
                            for ki, k in enumerate(g.ktail):
                                pt_ps = ptp.tile([P, P], f32, tag="pt")
                                nc.tensor.matmul(pt_ps[:],
                                                 ptt[:, ki * P:(ki + 1) * P],
                                                 ident[:], start=True, stop=True)
                                al_lo = g.AL[k][0]
                                nc.vector.tensor_copy(
                                    p_t[b][k][:, tcid * TC - al_lo:
                                              tcid * TC + TC - al_lo], pt_ps[:])
                    return run

                def pn_step(k):
                    def run():
                        al_lo, al_hi = g.AL[k]
                        pnk = pnp.tile([P, PW], bf16, tag="pn")
                        j0, j1 = al_lo // BANK, -(-al_hi // BANK)
                        for j in range(j0, j1):
                            lo = max(al_lo, j * BANK)
                            hi = min(al_hi, (j + 1) * BANK)
                            if j not in rb_t[b]:
                                rb = rbp.tile([P, BANK], f32, tag="rb")
                                nc.tensor.matmul(rb[:], e8[:, j * P:(j + 1) * P],
                                                 cur[b]["rr"][:],
                                                 start=True, stop=True)
                                rb_t[b][j] = rb
                            rb = rb_t[b][j]
                            nc.vector.tensor_tensor(
                                pnk[:, lo - al_lo:hi - al_lo],
                                p_t[b][k][:, lo - al_lo:hi - al_lo],
                                rb[:, lo - j * BANK:hi - j * BANK], op=ALU.mult)
                        pn_t[b][k] = pnk
                    return run

                lst = []
                pn_idx = {}
                ktail = set(g.ktail)
                # exp steps with pn trailing by one; tail before pn of ktail
                pend_pn = []
                for k in range(KN):
                    lst.append(e_step(k))
                    pend_pn.append(k)
                    while len(pend_pn) > 1:
                        kk = pend_pn[0]
                        if kk in ktail:
                            break
                        pn_idx[kk] = len(lst)
                        lst.append(pn_step(kk))
                        pend_pn.pop(0)
                if g.tail_chunks:
                    lst.append(tail_step())
                for kk in pend_pn:
                    pn_idx[kk] = len(lst)
                    lst.append(pn_step(kk))
                # step index needed before each output group tg
                req = []
                for tg in range(NTG):
                    ks = set()
                    for h in range(tg * (TG // HB), (tg + 1) * (TG // HB)):
                        ks |= set(g.win[h])
                    req.append(max(pn_idx[k] for k in ks) + 1)
                return lst, req

            def ensure_steps(b, upto):
                while nstep[b] < min(upto, len(steps[b])):
                    steps[b][nstep[b]]()
                    nstep[b] += 1

            reqs = [None] * BPC
            cur[0] = emit_input_dmas(0)
            for b in range(BPC):
                g = geoms[b]
                p_t[b] = [None] * KN
                pn_t[b] = [None] * KN
                steps[b], reqs[b] = mk_steps(b)
            # ---------- pipelined schedule ----------
            cast_seq = ["v", "v", "s", "s"] * 16
            for b in range(BPC):
                g = geoms[b]
                if b + 1 < BPC:
                    cur[b + 1] = emit_input_dmas(b + 1)
                ci = 0
                gi = 0
                for tg in range(NTG):
                    ensure_steps(b, reqs[b][tg])
                    for dci in range(D // P):
                        # drip remaining softmax steps between groups:
                        # this batch's first, then the next batch's
                        if nstep[b] < len(steps[b]):
                            ensure_steps(b, nstep[b] + 1)
                        elif b + 1 < BPC and gi % 2 == 1:
                            ensure_steps(b + 1, nstep[b + 1] + 1)
                        gi += 1
                        hid_t = cur[b]["hid"]
                        osb = osbp.tile([P, TG], bf16, tag="osb")
                        for bq in range(TG // BANK):
                            po = pop.tile([P, BANK], f32, tag="po")
                            h0 = (tg * (TG // BANK) + bq) * (BANK // HB)
                            if g.win[h0] == g.win[h0 + 1]:
                                # both halves share the window: 512-wide MMs
                                ks = g.win[h0]
                                for ki, k in enumerate(ks):
                                    al_lo = g.AL[k][0]
                                    nc.tensor.matmul(
                                        po[:],
                                        hid_t[:, k, dci * P:(dci + 1) * P],
                                        pn_t[b][k][:, h0 * HB - al_lo:
                                                   (h0 + 2) * HB - al_lo],
                                        start=(ki == 0),
                                        stop=(ki == len(ks) - 1))
                            else:
                                for hl in range(BANK // HB):
                                    h = h0 + hl
                                    ks = g.win[h]
                                    for ki, k in enumerate(ks):
                                        al_lo = g.AL[k][0]
                                        nc.tensor.matmul(
                                            po[:, hl * HB:(hl + 1) * HB],
                                            hid_t[:, k, dci * P:(dci + 1) * P],
                                            pn_t[b][k][:, h * HB - al_lo:
                                                       (h + 1) * HB - al_lo],
                                            start=(ki == 0),
                                            stop=(ki == len(ks) - 1))
                            dst = osb[:, bq * BANK:(bq + 1) * BANK]
                            eng = cast_seq[ci % len(cast_seq)]
                            ci += 1
                            if eng == "s":
                                nc.scalar.copy(dst, po[:])
                            else:
                                nc.vector.tensor_copy(dst, po[:])
                        nc.sync.dma_start(
                            out_d[b, dci * P:(dci + 1) * P,
                                  tg * TG:(tg + 1) * TG], osb[:])
    return nc


def _run(inputs, trace=False):
    import ml_dtypes

    import concourse.bacc as bacc
    from concourse.bass_utils import run_bass_kernel_spmd

    hidden = np.asarray(inputs["hidden"], dtype=np.float32)
    duration = np.asarray(inputs["duration"], dtype=np.float32)

    geoms, consts, arrays = _host_prep(duration)
    hid_bf = hidden.astype(ml_dtypes.bfloat16)

    nc = bacc.Bacc("TRN2", target_bir_lowering=False, debug=False,
                   enable_asserts=False, num_devices=NCORES)
    _build(nc, geoms, consts)
    nc.compile()

    in_maps = []
    for i in range(NCORES):
        sl = slice(i * BPC, (i + 1) * BPC)
        in_maps.append({
            "hidden": np.ascontiguousarray(hid_bf[sl]),
            "cTn": np.ascontiguousarray(arrays["cTn"][sl]),
            "rr": np.ascontiguousarray(arrays["rr"][sl]),
            "cbw": np.ascontiguousarray(arrays["cbw"][sl]),
            "mtail": np.ascontiguousarray(arrays["mtail"][sl]),
            "e8": arrays["e8"],
            "st16": arrays["st16"],
        })
    res = run_bass_kernel_spmd(nc, in_maps, core_ids=list(range(NCORES)),
                               trace=trace)
    out = np.concatenate([res.results[i]["out"] for i in range(NCORES)], axis=0)
    out = out.astype(np.float32)
    return out, res


def kernel(**inputs) -> np.ndarray:
    out, _ = _run(inputs, trace=False)
    return out
